# revision 1
# baseline (speedup 1.0000x reference)
"""Trainium2 Bass kernel for nn_JResCOPAttn (B=1, L=1024, D=128).

Reference computation:
    a   = x @ Wl.T + bl                        # [L, D]
    tm  = (a[:,None,:] * a[None,:,:]) @ Wlo.T + blo    # [L, L, D]  (never materialized!)
    tm *= (mask != 0)
    tx  = x @ Wl2.T + bl2                      # [L, D]
    y   = x + einsum('cad,ad->cd', tm, tx)
    out = LayerNorm(y) * gamma + beta

Algebraic restructuring used here (per output row c):
    y1[c,d] = sum_e act[c,e] * WloT[e,d] * S_c[e,d]  +  blo[d] * Z[c,d]
    S_c[e,d] = sum_a (mask[c,a]*act[a,e]) * tx[a,d]      (8 accumulating matmuls)
    Z[c,d]   = sum_a mask[c,a] * tx[a,d]                 (one batch of matmuls)
This avoids materializing the 536MB tm tensor entirely.

Sharding: rows c are split across the 8 NeuronCores (128 rows each); x is
replicated so each core computes act/tx for all 1024 source rows locally.
"""

import os
import sys

for _p in ("/opt/trn_rl_repo", "/root/.axon_site/_ro/trn_rl_repo"):
    if os.path.isdir(_p) and _p not in sys.path:
        sys.path.insert(0, _p)

import numpy as np

import concourse.bass as bass
import concourse.tile as tile
from concourse import bacc, mybir
from concourse.bass_utils import run_bass_kernel_spmd
from concourse.masks import make_identity

B, L, D = 1, 1024, 128
NCORES = 8
CB = L // NCORES          # c-rows per core = 128
T = L // 128              # a-tiles = 8
EPS = 1e-5
FP = mybir.dt.float32

# how many of the 8 per-c mask-apply ops run on DVE (rest on ScalarE/ACT)
N_DVE_MASK = 5
QUAD = 4                  # c's sharing one PSUM bank / one G multiply


def build_nc():
    nc = bacc.Bacc("TRN2", target_bir_lowering=False)

    # ---- I/O ----
    xT   = nc.dram_tensor("xT",   [128, L], FP, kind="ExternalInput")    # x^T (feature-major)
    xTb  = nc.dram_tensor("xTb",  [128, CB], FP, kind="ExternalInput")   # this core's block of xT cols
    xrow = nc.dram_tensor("xrow", [CB, D], FP, kind="ExternalInput")     # this core's x rows (residual)
    mT   = nc.dram_tensor("mT",   [128, T, CB], FP, kind="ExternalInput")  # mT[p,t,c] = mask[c0+c, t*128+p]
    WlT  = nc.dram_tensor("WlT",  [128, 128], FP, kind="ExternalInput")  # Wl.T
    Wl2T = nc.dram_tensor("Wl2T", [128, 128], FP, kind="ExternalInput")  # Wl2.T
    Wlo4 = nc.dram_tensor("Wlo4", [128, QUAD, 128], FP, kind="ExternalInput")  # Wlo.T replicated QUADx
    bl   = nc.dram_tensor("bl",   [128, 1], FP, kind="ExternalInput")
    bl2  = nc.dram_tensor("bl2",  [128, 1], FP, kind="ExternalInput")
    blo  = nc.dram_tensor("blo",  [128, 1], FP, kind="ExternalInput")
    gam  = nc.dram_tensor("gam",  [CB, D], FP, kind="ExternalInput")     # gamma broadcast to rows
    bet  = nc.dram_tensor("bet",  [CB, D], FP, kind="ExternalInput")
    out  = nc.dram_tensor("out",  [CB, D], FP, kind="ExternalOutput")

    Ident = mybir.ActivationFunctionType.Identity
    Sqrt = mybir.ActivationFunctionType.Sqrt
    mult = mybir.AluOpType.mult

    with tile.TileContext(nc) as tc:
        with (
            tc.tile_pool(name="singles", bufs=1) as singles,
            tc.tile_pool(name="trps", bufs=2, space="PSUM") as trps,
            tc.tile_pool(name="setps", bufs=2, space="PSUM") as setps,
            tc.tile_pool(name="ma", bufs=3) as ma_pool,
            tc.tile_pool(name="g", bufs=2) as g_pool,
            tc.tile_pool(name="s4", bufs=2, space="PSUM") as s4_pool,
            tc.tile_pool(name="y1tp", bufs=1, space="PSUM") as y1t_pool,
        ):
            # ---- load constants / inputs ----
            sb_xT = singles.tile([128, L], FP)
            nc.sync.dma_start(sb_xT, xT[:, :])
            sb_xTb = singles.tile([128, CB], FP)
            nc.sync.dma_start(sb_xTb, xTb[:, :])
            sb_xrow = singles.tile([CB, D], FP)
            nc.sync.dma_start(sb_xrow, xrow[:, :])
            sb_mT = singles.tile([128, T, CB], FP)
            nc.sync.dma_start(sb_mT, mT[:, :, :])
            sb_WlT = singles.tile([128, 128], FP)
            nc.sync.dma_start(sb_WlT, WlT[:, :])
            sb_Wl2T = singles.tile([128, 128], FP)
            nc.sync.dma_start(sb_Wl2T, Wl2T[:, :])
            sb_Wlo4 = singles.tile([128, QUAD, 128], FP)
            nc.sync.dma_start(sb_Wlo4, Wlo4[:, :, :])
            sb_bl = singles.tile([128, 1], FP)
            nc.sync.dma_start(sb_bl, bl[:, :])
            sb_bl2 = singles.tile([128, 1], FP)
            nc.sync.dma_start(sb_bl2, bl2[:, :])
            sb_blo = singles.tile([128, 1], FP)
            nc.sync.dma_start(sb_blo, blo[:, :])
            sb_gam = singles.tile([CB, D], FP)
            nc.sync.dma_start(sb_gam, gam[:, :])
            sb_bet = singles.tile([CB, D], FP)
            nc.sync.dma_start(sb_bet, bet[:, :])

            ident = singles.tile([128, 128], FP)
            make_identity(nc, ident)
            sb_eps = singles.tile([CB, 1], FP)
            nc.vector.memset(sb_eps, EPS)

            # ---- actT / txT = W @ xT + bias  (feature-major activations) ----
            actT = singles.tile([128, L], FP)
            txT = singles.tile([128, L], FP)
            for h in range(2):
                sl = slice(h * 512, (h + 1) * 512)
                ps_a = setps.tile([128, 512], FP, tag="set_mm")
                nc.tensor.matmul(ps_a, sb_WlT, sb_xT[:, sl], start=True, stop=True)
                nc.scalar.activation(actT[:, sl], ps_a, Ident, bias=sb_bl, scale=1.0)
                ps_t = setps.tile([128, 512], FP, tag="set_mm")
                nc.tensor.matmul(ps_t, sb_Wl2T, sb_xT[:, sl], start=True, stop=True)
                nc.scalar.activation(txT[:, sl], ps_t, Ident, bias=sb_bl2, scale=1.0)

            # actT restricted to this core's c-block (for the reduce matmuls)
            actTb = singles.tile([128, CB], FP)
            ps_b = setps.tile([128, 512], FP, tag="set_mm")
            nc.tensor.matmul(ps_b[:, :CB], sb_WlT, sb_xTb, start=True, stop=True)
            nc.scalar.activation(actTb, ps_b[:, :CB], Ident, bias=sb_bl, scale=1.0)

            # ---- natural-layout act / tx tiles via PE transpose ----
            act_nat = singles.tile([128, T, 128], FP)
            tx_nat = singles.tile([128, T, 128], FP)
            for t in range(T):
                sl = slice(t * 128, (t + 1) * 128)
                p1 = trps.tile([128, 128], FP, tag="tr")
                nc.tensor.transpose(p1, actT[:, sl], ident)
                nc.vector.tensor_copy(act_nat[:, t, :], p1)
                p2 = trps.tile([128, 128], FP, tag="tr")
                nc.tensor.transpose(p2, txT[:, sl], ident)
                nc.vector.tensor_copy(tx_nat[:, t, :], p2)

            # ---- ZT[d,c] = sum_a tx[a,d] * mask[c,a];  bloZT = blo * ZT ----
            zt_ps = setps.tile([128, 512], FP, tag="set_mm")
            for t in range(T):
                nc.tensor.matmul(
                    zt_ps[:, :CB], tx_nat[:, t, :], sb_mT[:, t, :],
                    start=(t == 0), stop=(t == T - 1),
                )
            bloZT = singles.tile([128, CB], FP)
            nc.vector.tensor_scalar_mul(bloZT, zt_ps[:, :CB], sb_blo)

            # ---- main loop over this core's 128 output rows ----
            y1t_ps = y1t_pool.tile([128, CB], FP)  # Y1^T columns, [d, c]
            for cq in range(CB // QUAD):
                s4 = s4_pool.tile([128, QUAD, 128], FP)
                for j in range(QUAD):
                    c = cq * QUAD + j
                    ma = ma_pool.tile([128, T, 128], FP, tag="ma")
                    for t in range(T):
                        if t < N_DVE_MASK:
                            nc.vector.tensor_scalar_mul(
                                ma[:, t, :], act_nat[:, t, :], sb_mT[:, t, c:c + 1]
                            )
                        else:
                            nc.scalar.mul(
                                ma[:, t, :], act_nat[:, t, :], sb_mT[:, t, c:c + 1]
                            )
                    for t in range(T):
                        nc.tensor.matmul(
                            s4[:, j, :], ma[:, t, :], tx_nat[:, t, :],
                            start=(t == 0), stop=(t == T - 1),
                        )
                g4 = g_pool.tile([128, QUAD, 128], FP, tag="g4")
                nc.vector.tensor_mul(g4, s4, sb_Wlo4)
                for j in range(QUAD):
                    c = cq * QUAD + j
                    nc.tensor.matmul(
                        y1t_ps[:, c:c + 1], g4[:, j, :], actTb[:, c:c + 1],
                        start=True, stop=True,
                    )

            # ---- combine, transpose back, residual, LayerNorm ----
            yt_sb = singles.tile([128, CB], FP)
            nc.vector.tensor_add(yt_sb, y1t_ps, bloZT)           # [d, c]
            y_ps = trps.tile([128, 128], FP, tag="tr")
            nc.tensor.transpose(y_ps, yt_sb, ident)              # [c, d]
            y_sb = singles.tile([CB, D], FP)
            nc.vector.tensor_add(y_sb, y_ps, sb_xrow)            # + x residual

            stats = singles.tile([CB, nc.vector.BN_STATS_DIM], FP)
            nc.vector.bn_stats(stats, y_sb)
            mv = singles.tile([CB, 2], FP)
            nc.vector.bn_aggr(mv, stats)
            nc.vector.tensor_scalar_sub(y_sb, y_sb, mv[:, 0:1])  # y - mean
            sd = singles.tile([CB, 1], FP)
            nc.scalar.activation(sd, mv[:, 1:2], Sqrt, bias=sb_eps, scale=1.0)
            rstd = singles.tile([CB, 1], FP)
            nc.vector.reciprocal(rstd, sd)
            nc.vector.tensor_scalar_mul(y_sb, y_sb, rstd)
            nc.vector.tensor_mul(y_sb, y_sb, sb_gam)
            nc.vector.tensor_add(y_sb, y_sb, sb_bet)

            nc.sync.dma_start(out[:, :], y_sb)

    return nc


_NC_CACHE = None


def _get_nc():
    global _NC_CACHE
    if _NC_CACHE is None:
        _NC_CACHE = build_nc()
        _NC_CACHE.finalize()
    return _NC_CACHE


def _prepare_in_maps(x, mask, Wl, bl, Wlo, blo, Wl2, bl2, gamma, beta):
    f32 = np.float32
    x0 = np.ascontiguousarray(np.asarray(x, f32)[0])          # [L, D]
    m = np.asarray(mask)[0].astype(f32)                       # [L, L] (c, a)
    xT = np.ascontiguousarray(x0.T)                           # [128, L]
    WlT = np.ascontiguousarray(np.asarray(Wl, f32).T)
    Wl2T = np.ascontiguousarray(np.asarray(Wl2, f32).T)
    WloT = np.ascontiguousarray(np.asarray(Wlo, f32).T)       # [e, d]
    Wlo4 = np.ascontiguousarray(
        np.broadcast_to(WloT[:, None, :], (128, QUAD, 128)).astype(f32)
    )
    bl_c = np.asarray(bl, f32).reshape(128, 1)
    bl2_c = np.asarray(bl2, f32).reshape(128, 1)
    blo_c = np.asarray(blo, f32).reshape(128, 1)
    gam_b = np.ascontiguousarray(np.broadcast_to(np.asarray(gamma, f32), (CB, D)))
    bet_b = np.ascontiguousarray(np.broadcast_to(np.asarray(beta, f32), (CB, D)))

    in_maps = []
    for k in range(NCORES):
        blk = slice(k * CB, (k + 1) * CB)
        mTk = m[blk, :].T.reshape(T, 128, CB).transpose(1, 0, 2)  # [p, t, c]
        in_maps.append({
            "xT": xT,
            "xTb": np.ascontiguousarray(xT[:, blk]),
            "xrow": np.ascontiguousarray(x0[blk]),
            "mT": np.ascontiguousarray(mTk),
            "WlT": WlT,
            "Wl2T": Wl2T,
            "Wlo4": Wlo4,
            "bl": bl_c,
            "bl2": bl2_c,
            "blo": blo_c,
            "gam": gam_b,
            "bet": bet_b,
        })
    return in_maps


def kernel(x, mask, Wl, bl, Wlo, blo, Wl2, bl2, gamma, beta):
    in_maps = _prepare_in_maps(x, mask, Wl, bl, Wlo, blo, Wl2, bl2, gamma, beta)
    res = run_bass_kernel_spmd(_get_nc(), in_maps, core_ids=list(range(NCORES)))
    y = np.concatenate([res.results[k]["out"] for k in range(NCORES)], axis=0)
    return y.reshape(B, L, D).astype(np.float32)



# revision 10
# speedup vs baseline: 1.3640x; 1.3640x over previous
"""Trainium2 Bass kernel for nn_JResCOPAttn (B=1, L=1024, D=128).

Reference computation:
    a   = x @ Wl.T + bl                        # [L, D]
    tm  = (a[:,None,:] * a[None,:,:]) @ Wlo.T + blo    # [L, L, D]  (never materialized!)
    tm *= (mask != 0)
    tx  = x @ Wl2.T + bl2                      # [L, D]
    y   = x + einsum('cad,ad->cd', tm, tx)
    out = LayerNorm(y) * gamma + beta

Algebraic restructuring used here (per output row c):
    y1[c,d] = sum_e act[c,e] * WloT[e,d] * S_c[e,d]  +  blo[d] * Z[c,d]
    S_c[e,d] = sum_a (mask[c,a]*act[a,e]) * tx[a,d]      (8 accumulating matmuls)
    Z[c,d]   = sum_a mask[c,a] * tx[a,d]                 (one batch of matmuls)
This avoids materializing the 536MB tm tensor entirely.

All heavy matmuls and elementwise mask ops run in bf16 (PE: 1 cycle/row vs 4
for fp32; DVE: 2x mode).  The per-c mask-apply is split across DVE (4 t-tiles
as tensor_scalar), GpSimd/Pool (3 t-tiles as one fused broadcast multiply) and
the Scalar engine (1 t-tile).  LayerNorm and the residual stay fp32.

Sharding: rows c are split across the 8 NeuronCores (128 rows each); x is
replicated so each core computes act/tx for all 1024 source rows locally.
"""

import os
import sys

for _p in ("/opt/trn_rl_repo", "/root/.axon_site/_ro/trn_rl_repo"):
    if os.path.isdir(_p) and _p not in sys.path:
        sys.path.insert(0, _p)

import numpy as np
import ml_dtypes

import concourse.bass as bass
import concourse.tile as tile
from concourse import bacc, mybir
from concourse.bass_utils import run_bass_kernel_spmd

B, L, D = 1, 1024, 128
NCORES = 8
CB = L // NCORES          # c-rows per core = 128
T = L // 128              # a-tiles = 8
EPS = 1e-5
FP = mybir.dt.float32
BF = mybir.dt.bfloat16
QUAD = 4                  # c's sharing one PSUM bank / one g multiply

# split of the 8 per-c mask-apply t-tiles across engines
DVE_T = (0, 1, 2)         # DVE: tensor_scalar_mul, bf16 2x mode
GP_LO, GP_HI = 3, 7       # GpSimd(Pool): one fused broadcast tensor_mul
SC_T = (7,)               # Scalar(Act): per-tile activation scale


def build_nc():
    nc = bacc.Bacc("TRN2", target_bir_lowering=False)

    # ---- I/O ----
    xT    = nc.dram_tensor("xT",    [128, L], BF, kind="ExternalInput")    # x^T bf16
    xTb   = nc.dram_tensor("xTb",   [128, CB], BF, kind="ExternalInput")   # this core's block of xT cols
    xrow  = nc.dram_tensor("xrow",  [CB, D], FP, kind="ExternalInput")     # this core's x rows (residual)
    mTb   = nc.dram_tensor("mTb",   [128, T, CB], BF, kind="ExternalInput")  # mTb[p,t,c] = mask[c0+c, t*128+p]
    mTf   = nc.dram_tensor("mTf",   [128, T, CB], FP, kind="ExternalInput")  # fp32 copy for scalar operands
    WlT   = nc.dram_tensor("WlT",   [128, 128], BF, kind="ExternalInput")  # Wl.T
    Wl2T  = nc.dram_tensor("Wl2T",  [128, 128], BF, kind="ExternalInput")  # Wl2.T
    Wlo4  = nc.dram_tensor("Wlo4",  [128, QUAD, 128], BF, kind="ExternalInput")  # Wlo.T replicated QUADx
    blrow = nc.dram_tensor("blrow", [1, 128], BF, kind="ExternalInput")    # bl as row (bias matmul)
    bl2row = nc.dram_tensor("bl2row", [1, 128], BF, kind="ExternalInput")
    bl    = nc.dram_tensor("bl",    [128, 1], FP, kind="ExternalInput")
    blo   = nc.dram_tensor("blo",   [128, 1], FP, kind="ExternalInput")
    gam   = nc.dram_tensor("gam",   [CB, D], FP, kind="ExternalInput")     # gamma broadcast to rows
    bet   = nc.dram_tensor("bet",   [CB, D], FP, kind="ExternalInput")
    out   = nc.dram_tensor("out",   [CB, D], FP, kind="ExternalOutput")

    Ident = mybir.ActivationFunctionType.Identity
    Sqrt = mybir.ActivationFunctionType.Sqrt

    with tile.TileContext(nc) as tc:
        with (
            tc.tile_pool(name="singles", bufs=1) as singles,
            tc.tile_pool(name="trps", bufs=2, space="PSUM") as trps,
            tc.tile_pool(name="setps", bufs=2, space="PSUM") as setps,
            tc.tile_pool(name="ma", bufs=4) as ma_pool,
            tc.tile_pool(name="g", bufs=2) as g_pool,
            tc.tile_pool(name="s4", bufs=2, space="PSUM") as s4_pool,
            tc.tile_pool(name="y1tp", bufs=1, space="PSUM") as y1t_pool,
        ):
            # ---- load constants / inputs ----
            sb_xT = singles.tile([128, L], BF)
            nc.sync.dma_start(sb_xT, xT[:, :])
            sb_xTb = singles.tile([128, CB], BF)
            nc.sync.dma_start(sb_xTb, xTb[:, :])
            sb_xrow = singles.tile([CB, D], FP)
            nc.sync.dma_start(sb_xrow, xrow[:, :])
            sb_mTb = singles.tile([128, T, CB], BF)
            nc.sync.dma_start(sb_mTb, mTb[:, :, :])
            sb_mTf = singles.tile([128, T, CB], FP)
            nc.sync.dma_start(sb_mTf, mTf[:, :, :])
            sb_WlT = singles.tile([128, 128], BF)
            nc.sync.dma_start(sb_WlT, WlT[:, :])
            sb_Wl2T = singles.tile([128, 128], BF)
            nc.sync.dma_start(sb_Wl2T, Wl2T[:, :])
            sb_Wlo4 = singles.tile([128, QUAD, 128], BF)
            nc.sync.dma_start(sb_Wlo4, Wlo4[:, :, :])
            sb_blrow = singles.tile([1, 128], BF)
            nc.sync.dma_start(sb_blrow, blrow[:, :])
            sb_bl2row = singles.tile([1, 128], BF)
            nc.sync.dma_start(sb_bl2row, bl2row[:, :])
            sb_bl = singles.tile([128, 1], FP)
            nc.sync.dma_start(sb_bl, bl[:, :])
            sb_blo = singles.tile([128, 1], FP)
            nc.sync.dma_start(sb_blo, blo[:, :])
            sb_gam = singles.tile([CB, D], FP)
            nc.sync.dma_start(sb_gam, gam[:, :])
            sb_bet = singles.tile([CB, D], FP)
            nc.sync.dma_start(sb_bet, bet[:, :])

            ones1 = singles.tile([1, 128], BF)
            nc.gpsimd.memset(ones1, 1.0)
            sb_eps = singles.tile([CB, 1], FP)
            nc.vector.memset(sb_eps, EPS)

            # ---- act/tx directly in natural [a, e] layout, bias via K=1 matmul ----
            act_nat = singles.tile([128, T, 128], BF)
            tx_nat = singles.tile([128, T, 128], BF)
            for t in range(T):
                sl = slice(t * 128, (t + 1) * 128)
                p1 = trps.tile([128, 128], FP, tag="tr")
                nc.tensor.matmul(p1, sb_xT[:, sl], sb_WlT, start=True, stop=False)
                nc.tensor.matmul(p1, ones1, sb_blrow, start=False, stop=True)
                nc.scalar.copy(act_nat[:, t, :], p1)
                p2 = trps.tile([128, 128], FP, tag="tr")
                nc.tensor.matmul(p2, sb_xT[:, sl], sb_Wl2T, start=True, stop=False)
                nc.tensor.matmul(p2, ones1, sb_bl2row, start=False, stop=True)
                nc.scalar.copy(tx_nat[:, t, :], p2)

            # actT restricted to this core's c-block (matvec moving operand)
            actTb = singles.tile([128, CB], BF)
            ps_b = setps.tile([128, CB], FP, tag="set_mm")
            nc.tensor.matmul(ps_b, sb_WlT, sb_xTb, start=True, stop=True)
            nc.scalar.activation(actTb, ps_b, Ident, bias=sb_bl, scale=1.0)

            # ---- ZT[d,c] = sum_a tx[a,d] * mask[c,a];  bloZT = blo * ZT ----
            zt_ps = setps.tile([128, CB], FP, tag="set_mm")
            for t in range(T):
                nc.tensor.matmul(
                    zt_ps, tx_nat[:, t, :], sb_mTb[:, t, :],
                    start=(t == 0), stop=(t == T - 1),
                )
            bloZT = singles.tile([128, CB], FP)
            nc.vector.tensor_scalar_mul(bloZT, zt_ps, sb_blo)

            # ---- main loop over this core's 128 output rows ----
            y1t_ps = y1t_pool.tile([128, CB], FP)  # Y1^T columns, [d, c]
            for cq in range(CB // QUAD):
                s4 = s4_pool.tile([128, QUAD, 128], FP)
                for j in range(QUAD):
                    c = cq * QUAD + j
                    ma = ma_pool.tile([128, T, 128], BF, tag="ma")
                    for t in DVE_T:
                        nc.vector.tensor_scalar_mul(
                            ma[:, t, :], act_nat[:, t, :], sb_mTf[:, t, c:c + 1]
                        )
                    nc.gpsimd.tensor_mul(
                        ma[:, GP_LO:GP_HI, :],
                        act_nat[:, GP_LO:GP_HI, :],
                        sb_mTb[:, GP_LO:GP_HI, c:c + 1].broadcast_to(
                            (128, GP_HI - GP_LO, 128)
                        ),
                    )
                    for t in SC_T:
                        nc.scalar.mul(
                            ma[:, t, :], act_nat[:, t, :], sb_mTf[:, t, c:c + 1]
                        )
                    for t in range(T):
                        nc.tensor.matmul(
                            s4[:, j, :], ma[:, t, :], tx_nat[:, t, :],
                            start=(t == 0), stop=(t == T - 1),
                        )
                g4 = g_pool.tile([128, QUAD, 128], BF, tag="g4")
                if cq % 2 == 0:
                    # DVE multiplies straight out of PSUM (fp32 in, bf16 out)
                    nc.vector.tensor_mul(g4, s4, sb_Wlo4)
                else:
                    # Scalar engine copies PSUM->SBUF bf16, DVE multiplies in
                    # SBUF at 2x; spreads the PSUM drain across two engines.
                    sg4 = g_pool.tile([128, QUAD, 128], BF, tag="sg4")
                    nc.scalar.copy(sg4, s4)
                    nc.vector.tensor_mul(g4, sg4, sb_Wlo4)
                for j in range(QUAD):
                    c = cq * QUAD + j
                    nc.tensor.matmul(
                        y1t_ps[:, c:c + 1], g4[:, j, :], actTb[:, c:c + 1],
                        start=True, stop=True,
                    )

            # ---- combine, transpose back, residual, LayerNorm ----
            from concourse.masks import make_identity
            ident = singles.tile([128, 128], FP)
            make_identity(nc, ident)

            yt_sb = singles.tile([128, CB], FP)
            nc.vector.tensor_add(yt_sb, y1t_ps, bloZT)           # [d, c]
            y_ps = trps.tile([128, 128], FP, tag="tr")
            nc.tensor.transpose(y_ps, yt_sb, ident)              # [c, d]
            y_sb = singles.tile([CB, D], FP)
            nc.vector.tensor_add(y_sb, y_ps, sb_xrow)            # + x residual

            stats = singles.tile([CB, nc.vector.BN_STATS_DIM], FP)
            nc.vector.bn_stats(stats, y_sb)
            mv = singles.tile([CB, 2], FP)
            nc.vector.bn_aggr(mv, stats)
            nc.vector.tensor_scalar_sub(y_sb, y_sb, mv[:, 0:1])  # y - mean
            sd = singles.tile([CB, 1], FP)
            nc.scalar.activation(sd, mv[:, 1:2], Sqrt, bias=sb_eps, scale=1.0)
            rstd = singles.tile([CB, 1], FP)
            nc.vector.reciprocal(rstd, sd)
            nc.vector.tensor_scalar_mul(y_sb, y_sb, rstd)
            nc.vector.tensor_mul(y_sb, y_sb, sb_gam)
            nc.vector.tensor_add(y_sb, y_sb, sb_bet)

            nc.sync.dma_start(out[:, :], y_sb)

    return nc


_NC_CACHE = None


def _get_nc():
    global _NC_CACHE
    if _NC_CACHE is None:
        _NC_CACHE = build_nc()
        _NC_CACHE.finalize()
    return _NC_CACHE


def _prepare_in_maps(x, mask, Wl, bl, Wlo, blo, Wl2, bl2, gamma, beta):
    f32 = np.float32
    bf16 = ml_dtypes.bfloat16
    x0 = np.ascontiguousarray(np.asarray(x, f32)[0])          # [L, D]
    m = np.asarray(mask)[0].astype(f32)                       # [L, L] (c, a)
    xT = np.ascontiguousarray(x0.T)                           # [128, L]
    WlT = np.ascontiguousarray(np.asarray(Wl, f32).T)
    Wl2T = np.ascontiguousarray(np.asarray(Wl2, f32).T)
    WloT = np.ascontiguousarray(np.asarray(Wlo, f32).T)       # [e, d]
    Wlo4 = np.ascontiguousarray(
        np.broadcast_to(WloT[:, None, :], (128, QUAD, 128))
    ).astype(bf16)
    bl_c = np.asarray(bl, f32).reshape(128, 1)
    blo_c = np.asarray(blo, f32).reshape(128, 1)
    blrow = np.asarray(bl, f32).reshape(1, 128).astype(bf16)
    bl2row = np.asarray(bl2, f32).reshape(1, 128).astype(bf16)
    gam_b = np.ascontiguousarray(np.broadcast_to(np.asarray(gamma, f32), (CB, D)))
    bet_b = np.ascontiguousarray(np.broadcast_to(np.asarray(beta, f32), (CB, D)))
    xT_bf = xT.astype(bf16)

    in_maps = []
    for k in range(NCORES):
        blk = slice(k * CB, (k + 1) * CB)
        mTk = m[blk, :].T.reshape(T, 128, CB).transpose(1, 0, 2)  # [p, t, c]
        mTk = np.ascontiguousarray(mTk)
        in_maps.append({
            "xT": xT_bf,
            "xTb": np.ascontiguousarray(xT_bf[:, blk]),
            "xrow": np.ascontiguousarray(x0[blk]),
            "mTb": mTk.astype(bf16),
            "mTf": mTk,
            "WlT": WlT.astype(bf16),
            "Wl2T": Wl2T.astype(bf16),
            "Wlo4": Wlo4,
            "blrow": blrow,
            "bl2row": bl2row,
            "bl": bl_c,
            "blo": blo_c,
            "gam": gam_b,
            "bet": bet_b,
        })
    return in_maps


def kernel(x, mask, Wl, bl, Wlo, blo, Wl2, bl2, gamma, beta):
    in_maps = _prepare_in_maps(x, mask, Wl, bl, Wlo, blo, Wl2, bl2, gamma, beta)
    res = run_bass_kernel_spmd(_get_nc(), in_maps, core_ids=list(range(NCORES)))
    y = np.concatenate([res.results[k]["out"] for k in range(NCORES)], axis=0)
    return y.reshape(B, L, D).astype(np.float32)
